# revision 22
# baseline (speedup 1.0000x reference)
"""Trainium2 Bass kernel for nn_Drifter (Euler integration of Fourier drift ODE).

Reference semantics:
    t = arange(0, 2001, 20) (T=101 points)
    drift(x) = sin(x*orders) @ sin_w + cos(x*orders) @ cos_w   (orders 0..7)
    x_{n+1} = x_n + drift(x_n) * 20
    xt[B, T] = all iterates wrapped to [-pi, pi);  t_mesh = broadcast t.

Device algorithm (turns space u = x/2pi, batch sharded 8 ways, 2 column
groups per core for cross-step pipelining):
    drift*DT/2pi = c0 + sum_{k=1..7} R_k sin(2pi(k u + B_k))
    state v_n = u_n - n*c0 (constant drift folded into per-step immediates)
    per step, per group:
      m_k = t - round(t), t = k*v + frac(B_k + k n c0)   [fused custom DVE ops,
                                    PageIdx pages k over runs (1-2),(3-5),(6-7);
                                    B_k offsets DMAed from a host table]
      s_k = Sin(2pi' m_k)  in waves, f32r output         [ACT]
      T   = sum_k R_k s_k  in PSUM                       [PE f32r matmuls; each
                            R_k split hi(10-bit exact) + lo so the amplitude
                            survives f32r weight truncation]
      v'  = v + T                                        [DVE stt, PSUM src]
    v_n is DMAed out time-major; host applies y = frac(v + n c0) (exact fp32,
    reference boundary convention) and the 2pi scale during the unshard
    transpose.
"""

import math

import numpy as np

B = 1048576
T = 101
T_STEPS = 100
NCORES = 8
BC = B // NCORES           # 131072 per core
P = 128
FT = BC // P               # 1024 free elements per partition per core
FS_ORDER = 8
NH = 7
DT = 20.0
TWO_PI = 2.0 * math.pi
MAGIC = 12582912.0         # 1.5 * 2**23: (x + M) - M == round-half-even(x)
SIN_SCALE = TWO_PI * (1.0 - 2.0 ** -22)  # keep spline arg strictly in [-pi, pi]

CFG = {
    "G": 2,
    "wave_sets": ((1,), (2, 3), (4, 5), (6, 7)),
    "merge": "f32r2",      # f32r2 | f32r1 | fp32
    "fp32_pe_set": (1, 2, 3, 4, 5, 6),
}

_LAST_NC = None
_op_cache = {}


def _get_custom_op():
    """Register (once) the fused phase+round+frac DVE op."""
    if "pf" in _op_cache:
        return _op_cache["pf"]
    import concourse.dve_ops as dve_ops
    from concourse.dve_spec import C0, C1, C2, Spec, Src0, lower
    from concourse.dve_uop import DveOpSpec

    name = "PHASE_FRAC_ANT"
    t = Src0 * C0 + C1
    q = (t + C2) - C2
    body = t - q

    def ref(in0, in1, s0, s1, imm2):
        tt = (in0 * np.float32(s0) + np.float32(s1)).astype(np.float32)
        qq = ((tt + np.float32(imm2)) - np.float32(imm2)).astype(np.float32)
        return (tt - qq).astype(np.float32)

    spec = Spec(body=body, reference=ref)

    def _register(name, spec, subdim, rd1):
        existing = {op.name: op for op in dve_ops.OPS}
        if name in existing:
            return existing[name]
        row = dve_ops._CUSTOM_DVE_ROW_BASE + len(dve_ops.OPS)
        shas = {}
        for ver in ("v3", "v4"):
            tmp = DveOpSpec(name=name, opcode=row, uops=lower(spec, ver=ver), rd1_en=rd1)
            shas[ver] = tmp.sha(ver)
        op = dve_ops.DveOp(name, spec, subdim=subdim, uops_sha=shas)
        dve_ops.OPS.append(op)
        dve_ops.CUSTOM_DVE_SPECS[name] = spec
        dve_ops._SUB_OPCODE_FOR_NAME[name] = row
        return op

    op = _register(name, spec, False, False)

    # fused all-harmonics op: in0 = v broadcast to [P, NH, Fg] (page-stride 0),
    # in1 = per-harmonic offsets broadcast along the free dim, k = 1 + page idx
    from concourse.dve_spec import One, PageIdx, Src1
    pg = PageIdx(C1, One)          # k = s1, s1+1, ... per page
    t7 = Src0 * pg + Src1
    q7 = (t7 + C0) - C0
    body7 = t7 - q7

    def ref7(in0, in1, s0, s1, imm2):
        i0 = np.asarray(in0, dtype=np.float32)
        i1 = np.asarray(in1, dtype=np.float32)
        # page count from the 3D view; harness passes the sliced operand
        S = i0.shape[1] if i0.ndim == 3 else 1
        i0 = i0.reshape(i0.shape[0], S, -1)
        i1 = i1.reshape(i1.shape[0], S, -1)
        k = (np.float32(s1) + np.arange(S, dtype=np.float32))[None, :, None]
        tt = (i0 * k + i1).astype(np.float32)
        qq = ((tt + np.float32(s0)) - np.float32(s0)).astype(np.float32)
        return (tt - qq).astype(np.float32).reshape(in0.shape)

    spec7 = Spec(body=body7, reference=ref7)
    op7 = _register("PHASE_FRAC7_ANT", spec7, True, True)
    _op_cache["pf"] = (op, op7)
    return _op_cache["pf"]


def _fold_weights(sin_weight, cos_weight):
    a = np.asarray(sin_weight, dtype=np.float64)
    b = np.asarray(cos_weight, dtype=np.float64)
    Rk = np.zeros(FS_ORDER)
    Bk = np.zeros(FS_ORDER)
    for k in range(1, FS_ORDER):
        Rk[k] = DT * math.hypot(a[k], b[k]) / TWO_PI
        Bk[k] = (math.atan2(b[k], a[k]) / TWO_PI) % 1.0
    c0 = DT * b[0] / TWO_PI
    return Rk, Bk, c0


def _trunc10(x):
    """Round x to 10 explicit mantissa bits (exact in any tf32-like format)."""
    f = np.float32(x)
    if f == 0:
        return f
    bits = f.view(np.int32)
    bits = np.int32(bits & ~np.int32((1 << 13) - 1))
    return bits.view(np.float32)


def _build_bass(Rk, Bk, c0):
    """Inputs: v0 [128, 1024] f32, wmat [128, NW*128] (f32r or f32).
    Output: y_tm [101, 131072] f32 — raw v states, time-major."""
    import concourse.bacc as bacc
    import concourse.mybir as mybir
    import concourse.tile as tile

    F32 = mybir.dt.float32
    F32R = mybir.dt.float32r
    ALU = mybir.AluOpType
    ACTF = mybir.ActivationFunctionType

    G = CFG["G"]
    Fg = FT // G
    wave_sets = CFG["wave_sets"]
    merge = CFG["merge"]
    f32r = True
    passes = 2 if merge == "f32r2" else 1
    pe_set = set(range(1, FS_ORDER))
    NW = passes * NH
    WD = F32R

    pf_op, pf7_op = _get_custom_op()

    nc = bacc.Bacc("TRN2", target_bir_lowering=False)
    v0_d = nc.dram_tensor("v0", [P, FT], F32, kind="ExternalInput")
    w_d = nc.dram_tensor("wmat", [P, NW * P], WD, kind="ExternalInput")
    out_d = nc.dram_tensor("y_tm", [T, BC], F32, kind="ExternalOutput")
    btab_d = nc.dram_tensor("btab", [T_STEPS, P, NH], F32, kind="ExternalInput")

    # weight slot index per (harmonic, pass)
    wslot_of = {}
    si = 0
    for k in sorted(pe_set):
        for p_ in range(passes):
            wslot_of[(k, p_)] = si
            si += 1

    def Bimm(k, n):
        return float((Bk[k] + k * ((n * c0) % 1.0)) % 1.0)

    with tile.TileContext(nc) as tc:
        with (
            tc.tile_pool(name="wpool", bufs=1) as wpool,
            tc.tile_pool(name="state", bufs=3) as state_pool,
            tc.tile_pool(name="mpool", bufs=2) as m_pool,
            tc.tile_pool(name="spool", bufs=2) as s_pool,
            tc.tile_pool(name="tpool", bufs=2) as t_pool,
            tc.tile_pool(name="psum", bufs=2, space="PSUM") as psum_pool,
        ):
            wtile = wpool.tile([P, NW, P], WD)
            nc.sync.dma_start(wtile[:], w_d[:].rearrange("p (h q) -> p h q", h=NW))

            v = []
            for g in range(G):
                vt = state_pool.tile([P, Fg], F32, tag=f"v{g}")
                nc.sync.dma_start(vt[:], v0_d[:, g * Fg:(g + 1) * Fg])
                v.append(vt[:])

            def emit_out(vap, g, n):
                dst = out_d[n].rearrange("(p f) -> p f", p=P)[:, g * Fg:(g + 1) * Fg]
                nc.sync.dma_start(dst, vap)

            for g in range(G):
                emit_out(v[g], g, 0)

            for n in range(T_STEPS):
                btile = t_pool.tile([P, NH], F32, tag="btab", name="btile")
                nc.sync.dma_start(btile[:], btab_d[n])
                for g in range(G):
                    vt = v[g]
                    m7 = m_pool.tile([P, NH, Fg], F32, tag=f"m{g}", name=f"m7_{g}")
                    s7 = s_pool.tile([P, NH, Fg], F32R, tag=f"s{g}", name=f"s7_{g}")

                    # fused custom ops: phases+round+frac per page-run
                    for lo, hi in CFG.get("pf_runs", ((0, 1), (1, 5), (5, 7))):
                        S = hi - lo
                        v_b = vt.unsqueeze(1).broadcast_to((P, S, Fg))
                        b_b = btile[:, lo:hi].unsqueeze(2).broadcast_to((P, S, Fg))
                        nc.vector._custom_dve(
                            pf7_op, out=m7[:, lo:hi], in0=v_b, in1=b_b,
                            s0=MAGIC, s1=float(lo + 1),
                        )

                    pt = psum_pool.tile([P, Fg], F32, tag=f"T{g}")
                    pe_seen = 0
                    npe = len(pe_set) * passes

                    done = 0
                    for wv in wave_sets:
                        lo, hi = done, done + len(wv)
                        nc.scalar.activation(
                            s7[:, lo:hi], m7[:, lo:hi], ACTF.Sin,
                            bias=0.0, scale=SIN_SCALE,
                        )
                        for k in wv:
                            for p_ in range(passes):
                                nc.tensor.matmul(
                                    pt[:], wtile[:, wslot_of[(k, p_)]], s7[:, k - 1],
                                    start=(pe_seen == 0), stop=(pe_seen == npe - 1),
                                )
                                pe_seen += 1
                        done = hi

                    # v' = v + T  (DVE stt, PSUM source)
                    vn = t_pool.tile([P, Fg], F32, tag=f"vn{g}", name=f"vn{g}")
                    nc.vector.scalar_tensor_tensor(
                        vn[:], pt[:], 1.0, vt, op0=ALU.mult, op1=ALU.add
                    )
                    v[g] = vn[:]

                    emit_out(v[g], g, n + 1)

    nc.compile()
    return nc


def _make_wmat(Rk):
    merge = CFG["merge"]
    f32r = merge.startswith("f32r")
    passes = 2 if merge == "f32r2" else 1
    pe_set = sorted(range(1, FS_ORDER)) if f32r else sorted(CFG["fp32_pe_set"])
    eye = np.eye(P, dtype=np.float32)
    blocks = []
    for k in pe_set:
        if f32r and passes == 2:
            hi = _trunc10(Rk[k])
            lo = np.float32(np.float64(Rk[k]) - np.float64(hi))
            blocks.append(eye * hi)
            blocks.append(eye * lo)
        else:
            blocks.append(eye * np.float32(Rk[k]))
    wmat = np.stack(blocks, axis=1)        # [P, NW, P]
    return np.ascontiguousarray(wmat.reshape(P, -1))


def kernel(x0_sample, sin_weight, cos_weight, t_sample):
    from concourse import bass_utils

    Rk, Bk, c0 = _fold_weights(sin_weight, cos_weight)
    nc = _build_bass(Rk, Bk, c0)
    global _LAST_NC
    _LAST_NC = nc

    x0 = np.asarray(x0_sample, dtype=np.float32)
    u0 = (x0.astype(np.float64) / TWO_PI).astype(np.float32)
    shards = u0.reshape(NCORES, P, FT)
    wmat = _make_wmat(Rk)
    btab = np.empty((T_STEPS, P, NH), dtype=np.float32)
    for n in range(T_STEPS):
        for k in range(1, FS_ORDER):
            btab[n, :, k - 1] = np.float32((Bk[k] + k * ((n * c0) % 1.0)) % 1.0)
    in_maps = [
        {"v0": np.ascontiguousarray(shards[c]), "wmat": wmat, "btab": btab}
        for c in range(NCORES)
    ]
    res = bass_utils.run_bass_kernel_spmd(nc, in_maps, core_ids=list(range(NCORES)))

    # host: y = frac_c(v + n*c0) with the reference wrap convention
    # (exact half-integers map to -0.5 i.e. -pi), then xt = 2pi * y^T.
    F = np.float32
    Cn = np.array([(n * c0) % 1.0 for n in range(T)], dtype=np.float32)[:, None]
    xt = np.empty((B, T), dtype=np.float32)
    for c in range(NCORES):
        v_tm = res.results[c]["y_tm"]                   # [T, BC] f32
        tp = (v_tm + Cn).astype(F)
        m = (tp - np.rint(tp)).astype(F)                # rint = round-half-even
        m = np.where(m == F(0.5), (m - F(1.0)).astype(F), m)
        xt[c * BC:(c + 1) * BC] = (m.T * F(TWO_PI)).astype(F)

    t = np.arange(0.0, 2001.0, DT, dtype=np.float32)
    t_mesh = np.broadcast_to(t[None, :], (B, T))
    return (t_mesh, xt)
